# revision 4
# baseline (speedup 1.0000x reference)
"""Trainium2 Bass kernel for nn_BandwidthPredictorNNHall.

Math: for each batch b (8 of them, one per NeuronCore) with particles
x [n=1024, d=4]:
    pilot_d = 1.0592 * std(x_d, ddof=1) * n^(-1/8)
    q = x / pilot,   K_ij = exp(-0.5 * |q_i - q_j|^2)
    s2_d = sum_ij K_ij ((q_jd - q_id)^2 - 1)
    bandwidth2 (s3 term) is exactly 0 (odd under i<->j).
With Mp = [1, p, p^2] (n x 9, RAW particle units), every sum needed for
s2 is an entry of V = Mp^T K Mp; the host applies ~30 scalar flops.

Device pipeline per core (v2: dual-engine exp stream + all-device tail):
  - One contiguous 16KB input DMA [128, 8, 4]; particle 8i+t ->
    (partition i, slot t) (any bijection is valid: all reductions are
    pair-permutation-invariant).
  - Stats on PE -> var -> s2 = 1/pilot^2 chain on DVE; 8 PE transposes
    stage p^T in PSUM; ScalarE+DVE split the PSUM->SBUF copy (QTrr);
    Pool builds scaled lhsT slices QTls = (s2*p)^T and the Mp tiles.
  - G = QTls^T @ QTrr per row tile I over columns [128 I, 1024) in wide
    f32r chunks (upper-triangle blocks only: 589k of 1M entries).
  - K'' = exp(G - r_i/2) = K_ij e^{+r_j/2}; row tiles alternate between
    ScalarE (true Exp activation, tiles 0/2/4/6) and DVE (tiles 1/3/5/7
    via a custom FAST_EXP op: out_i32 = relu(G*S + (B + nhall*S)), the
    int32 write bit-punned as f32 = Schraudolph exp, rel err ~2-3% per
    element, ~1e-3 on the final bandwidth after the pair sums). The two
    engines run concurrently, halving the exp-stream wall time.
  - Reduction without PE transposes: per column tile J,
        psW[:, J, :] += matmul(lhsT=K''-block(I,J), rhs=Mp[I])
    (0.5*Mp for the diagonal block), closing group J right after act_J;
    wsb_J = psW_J * cneg_J on Pool cancels the column scale; PE
    accumulates Vs += wsb_J^T Mp[J]. V = S + S^T with S = Vs (host f64).
  - Single output DMA: Vt = [Vs | stats columns] (stats give var on the
    host); no raw K-block offload, no separate var DMA.
"""

import sys

sys.path.insert(0, "/opt/trn_rl_repo")

import numpy as np

_B, _N, _D = 8, 1024, 4
_P = 128
_NT = _N // _P  # 8 row tiles
_NM = 1 + 2 * _D  # 9 basis columns: [1, p, p^2]
_INV_SQRT_2PI = 1.0 / np.sqrt(2.0 * np.pi)
_RK = 0.282095
_FACT = 1.0592 * float(_N) ** (-1.0 / (4 + _D))

# Schraudolph fast-exp constants (f32): bitcast(i32(relu(z*S + B))) ~ e^z
_SCH_S = float(np.float32(2.0**23 / np.log(2.0)))
_SCH_B = float(np.float32(127.0 * 2.0**23 - 361007.0))

# row tile I covers columns [128*I, 1024); KT column offset per tile
_W = [_N - _P * i for i in range(_NT)]
_OFF = [0] * _NT
for _i in range(1, _NT):
    _OFF[_i] = _OFF[_i - 1] + _W[_i - 1]
_KTW = sum(_W)  # 4608

_DVE_TILES = (1, 3, 5, 7)  # row tiles exp'd on DVE via FAST_EXP

_NC = None  # compiled Bass module cache
_FAST_EXP = None


def _register_fast_exp():
    """Register the FAST_EXP custom DVE op (relu(x*s0 + s1), int32 out)."""
    global _FAST_EXP
    if _FAST_EXP is not None:
        return _FAST_EXP
    import concourse.dve_ops as dops
    from concourse.dve_ops import DveOp
    from concourse.dve_spec import C0, C1, Spec, Src0, lower, relu
    from concourse.dve_uop import DveOpSpec

    name = "FAST_EXP_ANT"

    def _ref(in0, in1, s0, s1, imm2):
        return np.maximum(
            in0.astype(np.float32) * np.float32(s0) + np.float32(s1), 0.0
        )

    spec = Spec(body=relu(Src0 * C0 + C1), reference=_ref)
    if name in dops._SUB_OPCODE_FOR_NAME:
        _FAST_EXP = next(op for op in dops.OPS if op.name == name)
        return _FAST_EXP
    row = max(dops._SUB_OPCODE_FOR_NAME.values()) + 1
    assert row < 0x20
    dops._SUB_OPCODE_FOR_NAME[name] = row
    shas = {}
    for ver in ("v3", "v4"):
        s = DveOpSpec(name=name, opcode=row, uops=lower(spec, ver=ver), rd1_en=False)
        shas[ver] = s.sha(ver)
    op = DveOp(name, spec, subdim=False, uops_sha=shas)
    dops.OPS.append(op)
    dops.CUSTOM_DVE_SPECS[name] = spec
    _FAST_EXP = op
    return op


def _build_kernel():
    import concourse.bass as bass  # noqa: F401
    import concourse.tile as tile
    from concourse import bacc, mybir
    from concourse.masks import make_identity

    fast_exp = _register_fast_exp()

    f32 = mybir.dt.float32
    i32 = mybir.dt.int32
    fr = mybir.dt.float32r
    Act = mybir.ActivationFunctionType
    Alu = mybir.AluOpType
    Ax = mybir.AxisListType

    nc = bacc.Bacc("TRN2", target_bir_lowering=False, debug=False, num_devices=_B)
    p_in = nc.dram_tensor("p", [_N, _D], f32, kind="ExternalInput")
    s_out = nc.dram_tensor("sout", [_NM, _NM + 2], f32, kind="ExternalOutput")

    with tile.TileContext(nc) as tc:
        with (
            tc.tile_pool(name="singles", bufs=1) as singles,
            tc.tile_pool(name="psBig", bufs=2, space="PSUM") as psBig,
            tc.tile_pool(name="psW", bufs=1, space="PSUM") as psWp,
            tc.tile_pool(name="psV", bufs=1, space="PSUM") as psVp,
            tc.tile_pool(name="psLate", bufs=2, space="PSUM") as psLate,
        ):
            # ---- input DMA first: contiguous 16KB
            mstat = singles.tile([_P, _NT, _D], f32, tag="mstat")
            nc.sync.dma_start(
                out=mstat, in_=p_in[:].rearrange("(i r) d -> i r d", i=_P)
            )

            # ---- constants (Pool/DVE, overlap the DMA wait)
            ident128 = singles.tile([_P, _P], f32, tag="identf")
            make_identity(nc, ident128)
            ones128 = singles.tile([_P, 1], f32, tag="ones128")
            nc.vector.memset(ones128, 1.0)
            half4 = singles.tile([_D, _P], f32, tag="half4")
            nc.vector.memset(half4, 0.5)
            mtall = singles.tile([_P, _NT, _NM], f32, tag="mtall")
            nc.vector.memset(mtall[:, :, 0:1], 1.0)
            # dummy Exp so the activation-table load runs during the DMA wait
            warm = singles.tile([1, 1], f32, tag="warm")
            nc.scalar.activation(out=warm, in_=ones128[0:1, 0:1], func=Act.Exp)

            # ---- squares
            msq = singles.tile([_P, _NT, _D], f32, tag="msq")
            nc.vector.tensor_mul(msq, mstat, mstat)

            # ---- stats on PE: psS[:,0] = sum p, psS[:,1] = sum p^2
            psVm = psVp.tile([_P, 16], f32, tag="psvm")
            psS = psVm[0:_D, 9:11]
            for t in range(_NT):
                nc.tensor.matmul(
                    psS[:, 0:1], lhsT=mstat[:, t, :], rhs=ones128,
                    start=(t == 0), stop=(t == _NT - 1), skip_group_check=True,
                )
            for t in range(_NT):
                nc.tensor.matmul(
                    psS[:, 1:2], lhsT=msq[:, t, :], rhs=ones128,
                    start=(t == 0), stop=(t == _NT - 1), skip_group_check=True,
                )
            # feature-major rhs staging: 8 transposes into two psLate tiles
            psQa = psLate.tile([_P, 512], f32, name="psQa", tag="psl")
            psQb = psLate.tile([_P, 512], f32, name="psQb", tag="psl")
            for c in range(_NT):
                dst = psQa if c < 4 else psQb
                nc.tensor.transpose(
                    dst[0:_D, (c % 4) * _P : (c % 4 + 1) * _P],
                    mstat[:, c, :], ident128,
                )

            # ---- var chain (DVE): s2col = 1/pilot^2 as a [4,1] column
            sums = singles.tile([_D, 2], f32, tag="sums")
            nc.vector.tensor_copy(sums, psS)
            t1 = singles.tile([_D, 1], f32, tag="t1")
            nc.vector.tensor_scalar(
                out=t1, in0=sums[:, 0:1], scalar1=sums[:, 0:1],
                scalar2=-1.0 / _N, op0=Alu.mult, op1=Alu.mult,
            )
            den = singles.tile([_D, 1], f32, tag="den")
            nc.vector.tensor_add(den, t1, sums[:, 1:2])  # (n-1) var
            denf = singles.tile([_D, 1], f32, tag="denf")
            nc.vector.tensor_scalar_mul(denf, den, _FACT * _FACT / (_N - 1))
            s2col = singles.tile([_D, 1], f32, tag="s2col")
            nc.vector.reciprocal(s2col, denf)  # 1/pilot^2

            # ---- nhall = -r/2 per particle: diag(s2) via one DVE op, one
            # rank-1 PE broadcast, then multiply/reduce
            diag4 = singles.tile([_D, _D], f32, tag="diag4")
            nc.vector.tensor_scalar_mul(diag4, ident128[0:_D, 0:_D], s2col)
            psbc = psVm[:, 12:16]
            nc.tensor.matmul(
                psbc, lhsT=half4, rhs=diag4, start=True, stop=True,
                skip_group_check=True,
            )  # 0.5 * s2_d broadcast to all partitions
            scr = singles.tile([_P, _NT, _D], f32, tag="scr")
            nc.vector.tensor_mul(
                scr, msq, psbc.unsqueeze(1).broadcast_to((_P, _NT, _D))
            )
            nhall = singles.tile([_P, _NT], f32, tag="nhall")
            nc.vector.tensor_reduce(
                out=nhall, in_=scr, axis=Ax.X, op=Alu.add, negate=True
            )
            # Schraudolph per-partition bias for DVE tiles: sb1 = nhall*S + B
            sb1 = singles.tile([_P, _NT], f32, tag="sb1")
            nc.gpsimd.tensor_scalar(
                out=sb1, in0=nhall, scalar1=_SCH_S, scalar2=_SCH_B,
                op0=Alu.mult, op1=Alu.add,
            )
            cneg = singles.tile([_P, _NT], f32, tag="cneg")
            nc.scalar.activation(out=cneg, in_=nhall, func=Act.Exp)

            # ---- raw rhs QTrr = p^T [4, 1024]: split the PSUM->SBUF copy
            # across ScalarE and DVE
            QTrr = singles.tile([_D, _N], fr, tag="qtrr")
            nc.scalar.copy(QTrr[:, 0:_P], psQa[0:_D, 0:_P])
            nc.scalar.copy(QTrr[:, _P:512], psQa[0:_D, _P:512])
            nc.vector.tensor_copy(QTrr[:, 512:_N], psQb[0:_D, :])

            # ---- scaled lhsT slices on Pool; Mp tiles [1 | p | p^2]
            QTls = singles.tile([_D, _NT, _P], fr, tag="qtls")
            for c in range(2):
                nc.gpsimd.tensor_scalar_mul(
                    QTls[:, c, :], QTrr[:, c * _P : (c + 1) * _P], s2col
                )
            nc.gpsimd.tensor_copy(mtall[:, :, 1 : 1 + _D], mstat)
            nc.gpsimd.tensor_copy(mtall[:, :, 1 + _D : _NM], msq)
            mthalf = singles.tile([_P, _NT, _NM], f32, tag="mthalf")
            nc.gpsimd.tensor_scalar_mul(mthalf, mtall, 0.5)
            for c in range(2, _NT):
                nc.gpsimd.tensor_scalar_mul(
                    QTls[:, c, :], QTrr[:, c * _P : (c + 1) * _P], s2col
                )

            # ---- main stream: per row tile I (ascending), Gram chunks for
            # columns [128I, 1024) -> exp (ScalarE for even tiles, DVE
            # FAST_EXP for odd) -> per-block W matmuls; group J closes at
            # I==J, then wsb_J (Pool) and the Vs matmul fire.
            KT = singles.tile([_P, _KTW], f32, tag="kt")
            psW = psWp.tile([_P, _NT, _NM], f32, tag="psw")
            psVs = psVm[0:_NM, 0:_NM]
            wsb = singles.tile([_P, _NT, _NM], f32, tag="wsb")
            psg_t = [None] * _NT
            psg_base = [0, 0, 0, 0, 512, 640, 768, 896]

            def g_chunks(i):
                cs = _P * i
                if cs < 512:
                    return [(cs, 512), (512, _N)]
                return [(cs, _N)]

            def emit_g(i):
                # G7 (128 cols) rides in the tail of G5's tile with
                # start=False: G5's bank-zeroing start clears its region.
                if i < 4:
                    psg = psBig.tile([_P, _N], f32, tag="psg")
                elif i == 7:
                    psg = psBig.tile([_P, _P], f32, name="psg7", tag="psg")
                else:
                    psg = psLate.tile(
                        [_P, _N - psg_base[i]], f32, name=f"psl{i}", tag="psl"
                    )
                psg_t[i] = psg
                for a, b in g_chunks(i):
                    nc.tensor.matmul(
                        psg[:, a - psg_base[i] : b - psg_base[i]],
                        lhsT=QTls[:, i, :],
                        rhs=QTrr[:, a:b],
                        start=True, stop=True, skip_group_check=True,
                    )

            def emit_exp(i):
                src = psg_t[i][:, _P * i - psg_base[i] : _N - psg_base[i]]
                dst = KT[:, _OFF[i] : _OFF[i] + _W[i]]
                if i in _DVE_TILES:
                    nc.vector._custom_dve(
                        fast_exp, out=dst.bitcast(i32), in0=src,
                        s0=_SCH_S, s1=sb1[:, i : i + 1],
                    )
                else:
                    nc.scalar.activation(
                        out=dst, in_=src, func=Act.Exp, bias=nhall[:, i : i + 1]
                    )

            def emit_w(j):
                # group J = blocks (I <= J, J), contiguous in the psW bank
                for i in range(j + 1):
                    rhs = mthalf[:, i, :] if i == j else mtall[:, i, :]
                    nc.tensor.matmul(
                        psW[:, j, :],
                        lhsT=KT[:, _OFF[i] + _P * (j - i) : _OFF[i] + _P * (j - i + 1)],
                        rhs=rhs,
                        start=(i == 0), stop=(i == j), skip_group_check=True,
                    )

            def emit_wsb(j):
                # e^{-r_j/2} per partition cancels the K'' column scale.
                # PSUM source: only ScalarE/DVE can read it; split to
                # balance the two exp streams.
                if j in (1, 3):
                    nc.scalar.mul(wsb[:, j, :], psW[:, j, :], cneg[:, j : j + 1])
                else:
                    nc.vector.tensor_scalar_mul(
                        wsb[:, j, :], psW[:, j, :], cneg[:, j : j + 1]
                    )

            def emit_vsm(j):
                nc.tensor.matmul(
                    psVs, lhsT=wsb[:, j, :], rhs=mtall[:, j, :],
                    start=(j == 0), stop=(j == _NT - 1),
                )

            emit_g(0)
            emit_g(1)
            for i in range(_NT):
                emit_exp(i)
                if i + 2 < _NT:
                    emit_g(i + 2)
                emit_w(i)
                emit_wsb(i)
                # Vs matmuls deferred one iteration so they don't clog the
                # 4-deep PE wait queue while their wsb is pending
                if i > 0:
                    emit_vsm(i - 1)
            emit_vsm(_NT - 1)

            # ---- single output: Vt = [Vs | sum p | sum p^2] (host derives
            # var/pilot from the stats columns)
            Vt = singles.tile([_NM, _NM + 2], f32, tag="vt")
            nc.vector.tensor_copy(Vt, psVm[0:_NM, 0 : _NM + 2])
            nc.sync.dma_start(out=s_out[:], in_=Vt)

    nc.compile()
    return nc


def _get_nc():
    global _NC
    if _NC is None:
        _NC = _build_kernel()
    return _NC


def finalize(Sfull):
    """Host-side tail: Sfull [9, 11] = [S | sum p | sum p^2] -> bandwidth
    [4] in f64. V = S + S^T (block symmetry of the true K)."""
    Sfull = Sfull.astype(np.float64)
    S = Sfull[:, 0:_NM]
    sump = Sfull[0:_D, _NM]
    sumsq = Sfull[0:_D, _NM + 1]
    var = (sumsq - sump * sump / _N) / (_N - 1)
    pilot = _FACT * np.sqrt(var)
    V = S + S.T
    d = np.arange(_D)
    s2 = (
        (V[0, 5 + d] + V[5 + d, 0] - 2.0 * V[1 + d, 1 + d]) / pilot**2 - V[0, 0]
    ) * _INV_SQRT_2PI
    denom = _N * (_N - 1)
    I2 = s2 / pilot**5 / denom
    J1 = _RK / I2
    base = J1 / _N
    return (np.sign(base) * np.abs(base) ** 0.2).astype(np.float32)


def kernel(particles, weights=None, **_unused):
    from concourse.bass_utils import run_bass_kernel_spmd

    particles = np.ascontiguousarray(np.asarray(particles), dtype=np.float32)
    assert particles.shape == (_B, _N, _D), particles.shape

    nc = _get_nc()
    in_maps = [{"p": particles[c]} for c in range(_B)]
    res = run_bass_kernel_spmd(nc, in_maps, list(range(_B)))

    out = np.empty((_B, _D), np.float32)
    for c in range(_B):
        out[c] = finalize(res.results[c]["sout"])
    return out


# revision 8
# speedup vs baseline: 1.2149x; 1.2149x over previous
"""Trainium2 Bass kernel for nn_BandwidthPredictorNNHall.

Math: for each batch b (8, one per NeuronCore) with particles x [1024, 4]:
    pilot_d = 1.0592 * std(x_d, ddof=1) * n^(-1/8)
    q = x / pilot,   K_ij = exp(-0.5 |q_i - q_j|^2)
    s2_d = sum_ij K_ij ((q_jd - q_id)^2 - 1);  the s3 term is 0 (odd).
With Mp = [1, p, p^2] (n x 9, RAW units), every needed sum is an entry
of V = Mp^T K Mp; the host applies ~30 scalar flops.

v3 device pipeline per core:
  - 16KB input DMA [128, 8, 4] (particle 8i+t -> partition i, slot t).
  - Stats on PE -> var -> s2 = 1/pilot^2 on DVE; 8 PE transposes stage
    p^T; ScalarE+DVE split the PSUM->SBUF copy (QTrr); Pool builds the
    scaled lhsT slices QTls = (s2 p)^T and the Mp tiles.
  - Six PSUM banks form a 3-slot x [128,1024] rotation: psQ (transposes)
    -> G2 -> G5, G0 -> G3 -> G6, G1 -> G4 -> G7, so each Gram tile's
    bank is free exactly when its matmul is ready (no act-drain stalls).
  - K'' = exp(G - r_i/2) = K e^{+r_j/2}: even row tiles on ScalarE
    (true Exp), odd tiles on DVE via a custom FAST_EXP op
    (out_i32 = relu(G*S + (B + nhall*S)), int32 write bit-punned as f32
    = Schraudolph exp; ~2-3%/elem, ~1e-3 on the final bandwidth).
  - W reduction: psW[:, J, :] += K''-block(I,J)^T-free matmul with
    Mp[I] (0.5 Mp on the diagonal). Even J groups live in one bank,
    odd J in another, so the 1-deep PSUM group handoff (next group
    waits for the previous group's drain) never stalls the stream.
  - wsb_J = psW_J * cneg_J (cancels the column scale) on DVE/ScalarE;
    PE accumulates psVs += wsb_J^T Mp[J] for J<=6. wsb_7 ships raw in
    the output DMA; the host folds that last rank-9 term in f64.
  - Single output DMA: [Vs | sum p | sum p^2 | wsb7].
"""

import sys

sys.path.insert(0, "/opt/trn_rl_repo")

import numpy as np

_B, _N, _D = 8, 1024, 4
_P = 128
_NT = _N // _P  # 8 row tiles
_NM = 1 + 2 * _D  # 9 basis columns: [1, p, p^2]
_INV_SQRT_2PI = 1.0 / np.sqrt(2.0 * np.pi)
_RK = 0.282095
_FACT = 1.0592 * float(_N) ** (-1.0 / (4 + _D))

# Schraudolph fast-exp constants (f32): bitcast(i32(relu(z*S + B))) ~ e^z
_SCH_S = float(np.float32(2.0**23 / np.log(2.0)))
_SCH_B = float(np.float32(127.0 * 2.0**23 - 361007.0))

# row tile I covers columns [128*I, 1024); KT column offset per tile
_W = [_N - _P * i for i in range(_NT)]
_OFF = [0] * _NT
for _i in range(1, _NT):
    _OFF[_i] = _OFF[_i - 1] + _W[_i - 1]
_KTW = sum(_W)  # 4608

_DVE_TILES = (1, 3, 5, 7)  # row tiles exp'd on DVE via FAST_EXP

_NC = None  # compiled Bass module cache
_FAST_EXP = None


def _register_fast_exp():
    """Register the FAST_EXP custom DVE op (relu(x*s0 + s1), int32 out)."""
    global _FAST_EXP
    if _FAST_EXP is not None:
        return _FAST_EXP
    import concourse.dve_ops as dops
    from concourse.dve_ops import DveOp
    from concourse.dve_spec import C0, C1, Spec, Src0, lower, relu
    from concourse.dve_uop import DveOpSpec

    name = "FAST_EXP_ANT"

    def _ref(in0, in1, s0, s1, imm2):
        return np.maximum(
            in0.astype(np.float32) * np.float32(s0) + np.float32(s1), 0.0
        )

    spec = Spec(body=relu(Src0 * C0 + C1), reference=_ref)
    if name in dops._SUB_OPCODE_FOR_NAME:
        _FAST_EXP = next(op for op in dops.OPS if op.name == name)
        return _FAST_EXP
    row = max(dops._SUB_OPCODE_FOR_NAME.values()) + 1
    assert row < 0x20
    dops._SUB_OPCODE_FOR_NAME[name] = row
    shas = {}
    for ver in ("v3", "v4"):
        s = DveOpSpec(name=name, opcode=row, uops=lower(spec, ver=ver), rd1_en=False)
        shas[ver] = s.sha(ver)
    op = DveOp(name, spec, subdim=False, uops_sha=shas)
    dops.OPS.append(op)
    dops.CUSTOM_DVE_SPECS[name] = spec
    _FAST_EXP = op
    return op


def _build_kernel():
    import concourse.bass as bass  # noqa: F401
    import concourse.tile as tile
    from concourse import bacc, mybir
    from concourse.masks import make_identity

    fast_exp = _register_fast_exp()

    f32 = mybir.dt.float32
    i32 = mybir.dt.int32
    fr = mybir.dt.float32r
    Act = mybir.ActivationFunctionType
    Alu = mybir.AluOpType
    Ax = mybir.AxisListType

    nc = bacc.Bacc("TRN2", target_bir_lowering=False, debug=False, num_devices=_B)
    p_in = nc.dram_tensor("p", [_N, _D], f32, kind="ExternalInput")
    s_out = nc.dram_tensor("sout", [_P, 21], f32, kind="ExternalOutput")

    with tile.TileContext(nc) as tc:
        with (
            tc.tile_pool(name="singles", bufs=1) as singles,
            tc.tile_pool(name="psPair", bufs=3, space="PSUM") as psPair,
            tc.tile_pool(name="psWE", bufs=1, space="PSUM") as psWEp,
            tc.tile_pool(name="psVm", bufs=1, space="PSUM") as psVmp,
        ):
            # ---- input DMA first: contiguous 16KB
            mstat = singles.tile([_P, _NT, _D], f32, tag="mstat")
            nc.sync.dma_start(
                out=mstat, in_=p_in[:].rearrange("(i r) d -> i r d", i=_P)
            )

            # ---- constants (Pool/DVE, overlap the DMA wait)
            ident128 = singles.tile([_P, _P], f32, tag="identf")
            make_identity(nc, ident128)
            ones128 = singles.tile([_P, 1], f32, tag="ones128")
            nc.vector.memset(ones128, 1.0)
            half4 = singles.tile([_D, _P], f32, tag="half4")
            nc.vector.memset(half4, 0.5)
            mtall = singles.tile([_P, _NT, _NM], f32, tag="mtall")
            nc.vector.memset(mtall[:, :, 0:1], 1.0)
            # dummy Exp so the activation-table load runs during the DMA wait
            warm = singles.tile([1, 1], f32, tag="warm")
            nc.scalar.activation(out=warm, in_=ones128[0:1, 0:1], func=Act.Exp)

            # ---- squares
            msq = singles.tile([_P, _NT, _D], f32, tag="msq")
            nc.vector.tensor_mul(msq, mstat, mstat)

            vtout = singles.tile([_P, 21], f32, tag="vtout")
            nc.gpsimd.memset(vtout, 0.0)

            # psWE bank: W-even groups [0:36], psVs [36:45], psS [45:47]
            psWE = psWEp.tile([_P, 47], f32, tag="pswe")
            psS = psWE[0:_D, 45:47]
            psVs = psWE[0:_NM, 36:45]
            # psVm bank: psbc [0:4], W-odd groups [4:40]
            psVm = psVmp.tile([_P, 40], f32, tag="psvm")
            psbc = psVm[:, 0:4]

            def psw(j):
                if j % 2 == 0:
                    return psWE[:, 9 * (j // 2) : 9 * (j // 2) + 9]
                return psVm[:, 4 + 9 * (j // 2) : 13 + 9 * (j // 2)]

            # ---- stats on PE
            for t in range(_NT):
                nc.tensor.matmul(
                    psS[:, 0:1], lhsT=mstat[:, t, :], rhs=ones128,
                    start=(t == 0), stop=(t == _NT - 1), skip_group_check=True,
                )
            for t in range(_NT):
                nc.tensor.matmul(
                    psS[:, 1:2], lhsT=msq[:, t, :], rhs=ones128,
                    start=(t == 0), stop=(t == _NT - 1), skip_group_check=True,
                )
            # feature-major rhs staging: 8 transposes into rotation slot 0
            psQ = psPair.tile([_P, _N], f32, name="psQ", tag="psp")
            for c in range(_NT):
                nc.tensor.transpose(
                    psQ[0:_D, c * _P : (c + 1) * _P], mstat[:, c, :], ident128
                )

            # ---- var chain (DVE): s2col = 1/pilot^2 as a [4,1] column
            sums = singles.tile([_D, 2], f32, tag="sums")
            nc.vector.tensor_copy(sums, psS)
            t1 = singles.tile([_D, 1], f32, tag="t1")
            nc.vector.tensor_scalar(
                out=t1, in0=sums[:, 0:1], scalar1=sums[:, 0:1],
                scalar2=-1.0 / _N, op0=Alu.mult, op1=Alu.mult,
            )
            den = singles.tile([_D, 1], f32, tag="den")
            nc.vector.tensor_add(den, t1, sums[:, 1:2])  # (n-1) var
            denf = singles.tile([_D, 1], f32, tag="denf")
            nc.vector.tensor_scalar_mul(denf, den, _FACT * _FACT / (_N - 1))
            s2col = singles.tile([_D, 1], f32, tag="s2col")
            nc.vector.reciprocal(s2col, denf)  # 1/pilot^2
            nc.vector.tensor_copy(vtout[0:_D, 9:11], psS)

            # ---- nhall = -r/2: diag(s2) -> rank-1 PE broadcast -> mul/reduce
            diag4 = singles.tile([_D, _D], f32, tag="diag4")
            nc.vector.tensor_scalar_mul(diag4, ident128[0:_D, 0:_D], s2col)
            nc.tensor.matmul(
                psbc, lhsT=half4, rhs=diag4, start=True, stop=True,
                skip_group_check=True,
            )  # 0.5 * s2_d broadcast to all partitions

            # ---- raw rhs QTrr = p^T [4, 1024]: ScalarE does [0:512],
            # DVE does [512:1024]
            QTrr = singles.tile([_D, _N], fr, tag="qtrr")
            nc.scalar.copy(QTrr[:, 0:_P], psQ[0:_D, 0:_P])
            nc.scalar.copy(QTrr[:, _P:512], psQ[0:_D, _P:512])
            nc.vector.tensor_copy(QTrr[:, 512:_N], psQ[0:_D, 512:_N])

            # DVE tail of the pre-phase: scr -> nhall -> sb1 (keeps the
            # odd-tile FAST_EXP biases off the busy Pool engine)
            scr = singles.tile([_P, _NT, _D], f32, tag="scr")
            nc.vector.tensor_mul(
                scr, msq, psbc.unsqueeze(1).broadcast_to((_P, _NT, _D))
            )
            nhall = singles.tile([_P, _NT], f32, tag="nhall")
            nc.vector.tensor_reduce(
                out=nhall, in_=scr, axis=Ax.X, op=Alu.add, negate=True
            )
            sb1 = singles.tile([_P, _NT], f32, tag="sb1")
            nc.vector.tensor_scalar(
                out=sb1, in0=nhall, scalar1=_SCH_S, scalar2=_SCH_B,
                op0=Alu.mult, op1=Alu.add,
            )
            cneg = singles.tile([_P, _NT], f32, tag="cneg")
            nc.scalar.activation(out=cneg, in_=nhall, func=Act.Exp)

            # ---- scaled lhsT slices on Pool (0..3 first: they gate G0-G3);
            # Mp tiles in the middle (needed from W0 on)
            QTls = singles.tile([_D, _NT, _P], fr, tag="qtls")
            for c in range(4):
                nc.gpsimd.tensor_scalar_mul(
                    QTls[:, c, :], QTrr[:, c * _P : (c + 1) * _P], s2col
                )
            nc.gpsimd.tensor_copy(mtall[:, :, 1 : 1 + _D], mstat)
            nc.gpsimd.tensor_copy(mtall[:, :, 1 + _D : _NM], msq)
            mthalf = singles.tile([_P, _NT, _NM], f32, tag="mthalf")
            nc.gpsimd.tensor_scalar_mul(mthalf, mtall, 0.5)
            for c in range(4, _NT):
                nc.gpsimd.tensor_scalar_mul(
                    QTls[:, c, :], QTrr[:, c * _P : (c + 1) * _P], s2col
                )

            # ---- main stream
            KT = singles.tile([_P, _KTW], f32, tag="kt")
            wsb = singles.tile([_P, _NT - 1, _NM], f32, tag="wsb")
            psg_t = [None] * _NT

            def emit_g(i):
                # rotation: psQ,G0,G1 -> G2,G3,G4 -> G5,G6,G7
                psg = psPair.tile([_P, _N], f32, name=f"psg{i}", tag="psp")
                psg_t[i] = psg
                # matmul outputs must not cross the PSUM bank boundary at
                # tile-relative col 512
                w = _W[i]
                chunks = [(0, 512), (512, w)] if w > 512 else [(0, w)]
                for a, b in chunks:
                    nc.tensor.matmul(
                        psg[:, a:b],
                        lhsT=QTls[:, i, :],
                        rhs=QTrr[:, _P * i + a : _P * i + b],
                        start=True, stop=True, skip_group_check=True,
                    )

            def emit_exp(i):
                src = psg_t[i][:, 0 : _W[i]]
                dst = KT[:, _OFF[i] : _OFF[i] + _W[i]]
                if i in _DVE_TILES:
                    nc.vector._custom_dve(
                        fast_exp, out=dst.bitcast(i32), in0=src,
                        s0=_SCH_S, s1=sb1[:, i : i + 1],
                    )
                else:
                    nc.scalar.activation(
                        out=dst, in_=src, func=Act.Exp, bias=nhall[:, i : i + 1]
                    )

            def emit_w(j):
                # group J = blocks (I <= J, J); even J in psWE, odd in psVm
                for i in range(j + 1):
                    rhs = mthalf[:, i, :] if i == j else mtall[:, i, :]
                    nc.tensor.matmul(
                        psw(j),
                        lhsT=KT[:, _OFF[i] + _P * (j - i) : _OFF[i] + _P * (j - i + 1)],
                        rhs=rhs,
                        start=(i == 0), stop=(i == j), skip_group_check=True,
                    )

            def emit_wsb(j):
                # e^{-r_j/2} per partition cancels the K'' column scale
                dst = vtout[:, 12:21] if j == _NT - 1 else wsb[:, j, :]
                if j in (1, 3):
                    nc.scalar.mul(dst, psw(j), cneg[:, j : j + 1])
                else:
                    nc.vector.tensor_scalar_mul(dst, psw(j), cneg[:, j : j + 1])

            emit_g(0)
            emit_g(1)
            for i in range(_NT):
                emit_exp(i)
                if i + 2 < _NT:
                    emit_g(i + 2)
                emit_w(i)
                emit_wsb(i)
            # psVs group (last group in the psWE bank, after W6 drains):
            # J <= 6 on device; the host folds wsb7^T Mp[7]
            for j in range(_NT - 1):
                nc.tensor.matmul(
                    psVs, lhsT=wsb[:, j, :], rhs=mtall[:, j, :],
                    start=(j == 0), stop=(j == _NT - 2),
                )

            # ---- single output DMA: [Vs | sum p | sum p^2 | wsb7]
            nc.scalar.copy(vtout[0:_NM, 0:_NM], psVs)
            nc.sync.dma_start(out=s_out[:], in_=vtout)

    nc.compile()
    return nc


def _get_nc():
    global _NC
    if _NC is None:
        _NC = _build_kernel()
    return _NC


def finalize(raw, p):
    """Host tail: raw [128, 21] = [Vs(9x9) | sum p | sum p^2 | wsb7(128x9)]
    -> bandwidth [4] in f64. V = S + S^T; S = Vs + wsb7^T Mp[7]."""
    raw = raw.astype(np.float64)
    S = raw[0:_NM, 0:_NM].copy()
    sump = raw[0:_D, 9]
    sumsq = raw[0:_D, 10]
    wsb7 = raw[:, 12:21]
    p7 = p.astype(np.float64)[7::8]  # tile 7 particles
    M7 = np.concatenate([np.ones((_P, 1)), p7, p7 * p7], axis=1)
    S += wsb7.T @ M7
    var = (sumsq - sump * sump / _N) / (_N - 1)
    pilot = _FACT * np.sqrt(var)
    V = S + S.T
    d = np.arange(_D)
    s2 = (
        (V[0, 5 + d] + V[5 + d, 0] - 2.0 * V[1 + d, 1 + d]) / pilot**2 - V[0, 0]
    ) * _INV_SQRT_2PI
    denom = _N * (_N - 1)
    I2 = s2 / pilot**5 / denom
    base = _RK / I2 / _N
    return (np.sign(base) * np.abs(base) ** 0.2).astype(np.float32)


def kernel(particles, weights=None, **_unused):
    from concourse.bass_utils import run_bass_kernel_spmd

    particles = np.ascontiguousarray(np.asarray(particles), dtype=np.float32)
    assert particles.shape == (_B, _N, _D), particles.shape

    nc = _get_nc()
    in_maps = [{"p": particles[c]} for c in range(_B)]
    res = run_bass_kernel_spmd(nc, in_maps, list(range(_B)))

    out = np.empty((_B, _D), np.float32)
    for c in range(_B):
        out[c] = finalize(res.results[c]["sout"], particles[c])
    return out
